# revision 4
# baseline (speedup 1.0000x reference)
"""Conv2dfft kernel for Trainium2 (8 NeuronCores, SPMD data-parallel over N).

The reference computes an FFT-based 2D cross-correlation that is exactly a
3x3 same-padding conv2d: out[n,f,h,w] = sum_{c,ky,kx} x[n,c,h+ky-1,w+kx-1]
* weight[f,c,ky,kx] + bias[f]  (zero-padded at the borders).

We implement it directly as 9 shifted 128x128 matmuls accumulated in PSUM:
the contraction dim C=128 fills the PE partition dim, F=128 fills the output
partition dim. Data-parallel: 32 images / 8 cores = 4 images per core.

Trace-driven optimizations (NTFF analysis):
- bf16 inputs/outputs: the PE runs bf16 at 1 column/cycle (same as fp32r)
  but DMA bytes halve; fp32 accumulation in PSUM keeps rel err ~2e-3 vs
  the 2e-2 gate.
- DMA-dependency-free warmup matmuls on a gpsimd-iota'd tile: the PE
  starts ~2.5us before the input DMAs land AND the varying operand bits
  draw real PE power, which is what ramps the HAM clock 1.2->2.4GHz
  (all-zero warmup data demonstrably does not ramp it).
- DMA issue order: first-tap weights -> x chunk 0 -> rest of weights ->
  bias -> x1..x7, so the first real matmul's operands land first.
- NRT appends a per-execution teardown that clears semaphores
  [runtime_semaphore_count, 256) one EVENT_SEMAPHORE per sem, split
  across engines (~6.5us, fully inside the measured exec window). We
  clear [3, 250) ourselves with two 54ns EVENT_SEMAPHORE_RANGE_CLEARs in
  the kernel epilogue and patch runtime_semaphore_count=250 into the
  NEFF so NRT only clears [250, 256).
- Only the SP HWDGE dynamic-DMA queue group (8 queues) is declared
  instead of bass's default 3 groups x 16 (less NRT queue setup).
"""

import io
import tarfile
import tempfile

import numpy as np
import ml_dtypes
import orjson

import concourse.bass as bass
import concourse.bass2jax as bass2jax
import concourse.tile as tile
from concourse import bacc, mybir, neff as neff_mod
from concourse.bass_utils import run_bass_kernel_spmd

N, C, F, H, W = 32, 128, 128, 32, 32
N_CORES = 8
N_LOC = N // N_CORES  # images per core
HP, WP = H + 2, W + 2  # host-padded image
HB = 16      # rows per PSUM block (16*32 = 512 = one PSUM bank)
HC = HB + 2  # rows per x chunk (chunk hb covers padded rows 16*hb .. +18)
N_WARM = 5   # DMA-free warmup matmuls (PE clock ramp + DMA bridge)

# NRT clears semaphores [runtime_semaphore_count, 256) after every
# execution, one instruction per semaphore. Our epilogue range-clears
# [3, RT_SEM_COUNT) in two instructions instead.
RT_SEM_COUNT = 250

F32 = mybir.dt.float32
BF16 = mybir.dt.bfloat16


def _light_drain_and_barrier(self, tick_clock, wait_clock):
    """Tile epilogue without the trailing all-engine barrier, plus bulk
    semaphore range-clears standing in for NRT's per-sem teardown."""
    from concourse.vector_clock import ScopedClock

    drain_inst = self.nc.sync.drain()
    wait_clock.add_sem_waits(
        drain_inst.ins, ScopedClock({None: tick_clock.global_clock})
    )
    self.nc.all_engine_barrier()
    popped = self.nc._tile_sem_poison_stack.pop()
    assert popped is self._sem_poison
    self.nc.clear_and_free_semaphores(list(self.sems.allocated().values()))
    # All engines are quiescent after the barrier above; bulk-clear the
    # compiler/kernel semaphore space NRT will no longer touch (we patch
    # runtime_semaphore_count=RT_SEM_COUNT into the NEFF). The tile sems
    # were already cleared by clear_and_free_semaphores; re-clearing is
    # harmless and keeps the ranges simple.
    for rng in (range(3, 150), range(150, RT_SEM_COUNT)):
        self.nc.gpsimd.dma_reset(rng)
        self.nc.gpsimd.sem_clear(rng)


def _patch_neff_rt_sem_count(neff_path: str, mapping: dict) -> bytes:
    """rename_neff_tensors_and_patch_header + runtime_semaphore_count bump."""
    with tempfile.TemporaryDirectory() as repack_dir:
        with open(neff_path, "rb") as neff_f:
            old_neff_header = neff_f.read(1024)
            with tarfile.open(fileobj=neff_f, mode="r") as neff_tar:
                neff_tar.extractall(repack_dir)

        with open(f"{repack_dir}/neff.json") as f:
            neff_json = orjson.loads(f.read())
        for node in neff_json["nodes"]:
            node["name"] = mapping.get(node["name"], node["name"])
            node["output_names"] = [
                mapping.get(name, name) for name in node["output_names"]
            ]
        with open(f"{repack_dir}/neff.json", "w") as f:
            f.write(orjson.dumps(neff_json).decode())

        with open(f"{repack_dir}/sg00/def.json") as f:
            def_json = orjson.loads(f.read())
        def_json["var"] = {
            mapping.get(name, name): items for name, items in def_json["var"].items()
        }
        def_json["runtime_semaphore_count"] = RT_SEM_COUNT
        with open(f"{repack_dir}/sg00/def.json", "w") as f:
            f.write(orjson.dumps(def_json).decode())

        buf = io.BytesIO()
        with tarfile.open(fileobj=buf, mode="w") as neff_tar:
            neff_tar.add(repack_dir, arcname=".", filter=bass2jax._reset_tarinfo)
        new_neff_data = buf.getvalue()
        new_neff_header = neff_mod.make_deterministic_neff_header(
            old_neff_header=old_neff_header,
            new_neff_data=new_neff_data,
        )
    return new_neff_header + new_neff_data


bass2jax.rename_neff_tensors_and_patch_header = _patch_neff_rt_sem_count


def _build_module():
    nc = bacc.Bacc(None, dynamic_dma_scratch_size=256)

    # x is stored as per-block 18-row chunks (rows 16*hb .. 16*hb+18 of the
    # padded image), so each (image, hb) PSUM block reads exactly one chunk.
    x_d = nc.dram_tensor(
        "x", [N_LOC, H // HB, C, HC, WP], BF16, kind="ExternalInput"
    )
    w_d = nc.dram_tensor("w", [C, 9 * F], BF16, kind="ExternalInput")
    b_d = nc.dram_tensor("b", [F, 1], F32, kind="ExternalInput")
    o_d = nc.dram_tensor("out", [N_LOC, F, H, W], BF16, kind="ExternalOutput")

    n_blocks = N_LOC * (H // HB)

    tile.TileContext._drain_and_barrier = _light_drain_and_barrier
    with tile.TileContext(nc) as tc:
        with (
            tc.tile_pool(name="const", bufs=1) as cpool,
            tc.tile_pool(name="x", bufs=n_blocks) as xpool,
            tc.tile_pool(name="o", bufs=n_blocks) as opool,
            tc.tile_pool(name="ps", bufs=8, space=bass.MemorySpace.PSUM) as ppool,
        ):
            # DMA-free warmups on varying (iota) data: no DMA dependency, and
            # the operand bit-toggling draws PE power so the HAM clock ramps
            # 1.2GHz -> 2.4GHz before the real matmuls issue.
            wu = cpool.tile([C, 512], BF16)
            nc.gpsimd.iota(
                wu[:],
                pattern=[[1, 512]],
                channel_multiplier=3,
                allow_small_or_imprecise_dtypes=True,
            )
            ps_warm = ppool.tile([F, HB, W], F32, tag="ps")
            prev_mm = None
            for i in range(N_WARM):
                prev_mm = nc.tensor.matmul(
                    ps_warm[:],
                    wu[:, 0:F],
                    wu[:],
                    start=True,
                    stop=True,
                )

            # First-tap weights first, then x chunk 0: the first real
            # matmul's operands are first in the serial SP trigger queue.
            w_sb = cpool.tile([C, 9 * F], BF16)
            nc.sync.dma_start(w_sb[:, 0:F], w_d[:, 0:F])

            x_sbs = []
            x_sb = xpool.tile([C, HC, WP], BF16, tag="x")
            nc.sync.dma_start(x_sb[:], x_d[0, 0])
            x_sbs.append(x_sb)

            nc.sync.dma_start(w_sb[:, F : 9 * F], w_d[:, F : 9 * F])

            b_sb = cpool.tile([F, 1], F32)
            nc.sync.dma_start(b_sb[:], b_d[:])

            for blk in range(1, n_blocks):
                n, hb = divmod(blk, H // HB)
                x_sb = xpool.tile([C, HC, WP], BF16, tag="x")
                nc.sync.dma_start(x_sb[:], x_d[n, hb])
                x_sbs.append(x_sb)

            for n in range(N_LOC):
                for hb in range(H // HB):
                    x_sb = x_sbs[n * (H // HB) + hb]
                    ps = ppool.tile([F, HB, W], F32, tag="ps")
                    for i, (ky, kx) in enumerate(
                        [(ky, kx) for ky in range(3) for kx in range(3)]
                    ):
                        rhs = x_sb[:, ky : ky + HB, kx : kx + W]
                        lhsT = w_sb[:, (ky * 3 + kx) * F : (ky * 3 + kx + 1) * F]
                        mm = nc.tensor.matmul(
                            ps[:],
                            lhsT,
                            rhs,
                            start=(i == 0),
                            stop=(i == 8),
                        )
                        if prev_mm is not None:
                            # keep PE issue order = program order
                            tile.add_dep_helper(
                                mm.ins, prev_mm.ins, sync=False,
                                reason="PE program order",
                            )
                        prev_mm = mm
                    # bias add PSUM -> SBUF (bf16), then store this block
                    o_sb = opool.tile([F, HB, W], BF16, tag="o")
                    nc.vector.tensor_scalar_add(o_sb[:], ps[:], b_sb[:, 0:1])
                    nc.sync.dma_start(o_d[n][:, hb * HB : hb * HB + HB, :], o_sb[:])
    nc.compile()

    # Declare only the DMA queue group we use (SP HWDGE), and fewer rings:
    # NRT's per-execution queue init/reset work scales with declarations.
    nc.m.queues = [q for q in nc.m.queues if q.name == "qSPDynamicHW"]
    for q in nc.m.queues:
        q.num_queues = 8
    return nc


_NC_CACHE = None


def _run(x, weight, bias, **kwargs):
    global _NC_CACHE
    if _NC_CACHE is None:
        _NC_CACHE = _build_module()
    nc = _NC_CACHE

    xp = np.zeros((N, C, HP, WP), dtype=ml_dtypes.bfloat16)
    xp[:, :, 1 : 1 + H, 1 : 1 + W] = np.asarray(x, dtype=np.float32).astype(
        ml_dtypes.bfloat16
    )
    # per-block 18-row chunks: chunk hb = padded rows 16*hb .. 16*hb+18
    xc = np.stack([xp[:, :, 0:HC, :], xp[:, :, HB : HB + HC, :]], axis=1)
    # lhsT layout: w_pack[c, (ky*3+kx)*F + f] = weight[f, c, ky, kx]
    w_pack = np.ascontiguousarray(
        np.asarray(weight, dtype=np.float32).transpose(1, 2, 3, 0).reshape(C, 9 * F)
    ).astype(ml_dtypes.bfloat16)
    b2 = np.ascontiguousarray(np.asarray(bias, dtype=np.float32).reshape(F, 1))

    shards = xc.reshape(N_CORES, N_LOC, H // HB, C, HC, WP)
    in_maps = [{"x": shards[i], "w": w_pack, "b": b2} for i in range(N_CORES)]
    return run_bass_kernel_spmd(nc, in_maps, core_ids=list(range(N_CORES)), **kwargs)


def kernel(x: np.ndarray, weight: np.ndarray, bias: np.ndarray, **_) -> np.ndarray:
    res = _run(x, weight, bias)
    return np.concatenate(
        [res.results[i]["out"].astype(np.float32) for i in range(N_CORES)], axis=0
    )


# revision 6
# speedup vs baseline: 1.1288x; 1.1288x over previous
"""Conv2dfft kernel for Trainium2 (8 NeuronCores, SPMD data-parallel over N).

The reference computes an FFT-based 2D cross-correlation that is exactly a
3x3 same-padding conv2d: out[n,f,h,w] = sum_{c,ky,kx} x[n,c,h+ky-1,w+kx-1]
* weight[f,c,ky,kx] + bias[f]  (zero-padded at the borders).

We implement it directly as 9 shifted 128x128 matmuls accumulated in PSUM:
the contraction dim C=128 fills the PE partition dim, F=128 fills the output
partition dim. Data-parallel: 32 images / 8 cores = 4 images per core.

Trace-driven optimizations (NTFF analysis):
- bf16 inputs/outputs: the PE runs bf16 at 1 column/cycle (same as fp32r)
  but DMA bytes halve; fp32 accumulation in PSUM keeps rel err ~3e-3 vs
  the 2e-2 gate.
- DMA-dependency-free warmup matmuls in *fp32* on a gpsimd-iota'd tile:
  the PE starts ~1.5us before the input DMAs land, and fp32 matmuls draw
  maximum PE power, which is what ramps the HAM clock 1.2->2.4GHz
  (bf16-only activity takes ~8us to ramp; fp32/fp32r takes ~4us).
- DMA issue order: first-tap weights -> x chunk 0 -> rest of weights ->
  bias -> x1..x7, so the first real matmul's operands land first.
- Only the SP HWDGE dynamic-DMA queue group (8 queues) is declared
  instead of bass's default 3 groups x 16 (less NRT queue setup).
"""

import numpy as np
import ml_dtypes

import concourse.bass as bass
import concourse.tile as tile
from concourse import bacc, mybir
from concourse.bass_utils import run_bass_kernel_spmd

N, C, F, H, W = 32, 128, 128, 32, 32
N_CORES = 8
N_LOC = N // N_CORES  # images per core
HP, WP = H + 2, W + 2  # host-padded image
HB = 16      # rows per PSUM block (16*32 = 512 = one PSUM bank)
HC = HB + 2  # rows per x chunk (chunk hb covers padded rows 16*hb .. +18)
N_WARM = 6   # DMA-free fp32 warmup matmuls (PE clock ramp + DMA bridge)

F32 = mybir.dt.float32
BF16 = mybir.dt.bfloat16


def _light_drain_and_barrier(self, tick_clock, wait_clock):
    """Tile epilogue without the trailing all-engine barrier.

    Nothing executes after the semaphore clears inside this kernel, and the
    runtime won't re-dispatch the NEFF until every engine queue has drained,
    so the final barrier only adds tail latency.
    """
    from concourse.vector_clock import ScopedClock

    drain_inst = self.nc.sync.drain()
    wait_clock.add_sem_waits(
        drain_inst.ins, ScopedClock({None: tick_clock.global_clock})
    )
    self.nc.all_engine_barrier()
    popped = self.nc._tile_sem_poison_stack.pop()
    assert popped is self._sem_poison
    self.nc.clear_and_free_semaphores(list(self.sems.allocated().values()))


def _build_module():
    nc = bacc.Bacc(None, dynamic_dma_scratch_size=256)

    # x is stored as per-block 18-row chunks (rows 16*hb .. 16*hb+18 of the
    # padded image), so each (image, hb) PSUM block reads exactly one chunk.
    x_d = nc.dram_tensor(
        "x", [N_LOC, H // HB, C, HC, WP], BF16, kind="ExternalInput"
    )
    w_d = nc.dram_tensor("w", [C, 9 * F], BF16, kind="ExternalInput")
    b_d = nc.dram_tensor("b", [F, 1], F32, kind="ExternalInput")
    o_d = nc.dram_tensor("out", [N_LOC, F, H, W], BF16, kind="ExternalOutput")

    n_blocks = N_LOC * (H // HB)

    tile.TileContext._drain_and_barrier = _light_drain_and_barrier
    with tile.TileContext(nc) as tc:
        with (
            tc.tile_pool(name="const", bufs=1) as cpool,
            tc.tile_pool(name="x", bufs=n_blocks) as xpool,
            tc.tile_pool(name="o", bufs=n_blocks) as opool,
            tc.tile_pool(name="ps", bufs=8, space=bass.MemorySpace.PSUM) as ppool,
        ):
            # DMA-free fp32 warmups on varying (iota) data: no DMA
            # dependency, and fp32 matmuls draw maximum PE power so the HAM
            # clock ramps to 2.4GHz before the real bf16 matmuls issue.
            wu = cpool.tile([C, 512], F32)
            nc.gpsimd.iota(
                wu[:],
                pattern=[[1, 512]],
                channel_multiplier=3001,
                allow_small_or_imprecise_dtypes=True,
            )
            ps_warm = ppool.tile([F, HB, W], F32, tag="ps")
            prev_mm = None
            for i in range(N_WARM):
                # 128-wide fp32 matmul: 512 PE cycles each, max power draw.
                prev_mm = nc.tensor.matmul(
                    ps_warm[:, 0:4, :],
                    wu[:, 0:F],
                    wu[:, (i % 4) * F : (i % 4) * F + F],
                    start=True,
                    stop=True,
                )

            # First-tap weights first, then x chunk 0: the first real
            # matmul's operands are first in the serial SP trigger queue.
            w_sb = cpool.tile([C, 9 * F], BF16)
            nc.sync.dma_start(w_sb[:, 0:F], w_d[:, 0:F])

            x_sbs = []
            x_sb = xpool.tile([C, HC, WP], BF16, tag="x")
            nc.sync.dma_start(x_sb[:], x_d[0, 0])
            x_sbs.append(x_sb)

            nc.sync.dma_start(w_sb[:, F : 9 * F], w_d[:, F : 9 * F])

            b_sb = cpool.tile([F, 1], F32)
            nc.sync.dma_start(b_sb[:], b_d[:])

            for blk in range(1, n_blocks):
                n, hb = divmod(blk, H // HB)
                x_sb = xpool.tile([C, HC, WP], BF16, tag="x")
                nc.sync.dma_start(x_sb[:], x_d[n, hb])
                x_sbs.append(x_sb)

            for n in range(N_LOC):
                for hb in range(H // HB):
                    x_sb = x_sbs[n * (H // HB) + hb]
                    ps = ppool.tile([F, HB, W], F32, tag="ps")
                    for i, (ky, kx) in enumerate(
                        [(ky, kx) for ky in range(3) for kx in range(3)]
                    ):
                        rhs = x_sb[:, ky : ky + HB, kx : kx + W]
                        lhsT = w_sb[:, (ky * 3 + kx) * F : (ky * 3 + kx + 1) * F]
                        mm = nc.tensor.matmul(
                            ps[:],
                            lhsT,
                            rhs,
                            start=(i == 0),
                            stop=(i == 8),
                        )
                        if prev_mm is not None:
                            # keep PE issue order = program order
                            tile.add_dep_helper(
                                mm.ins, prev_mm.ins, sync=False,
                                reason="PE program order",
                            )
                        prev_mm = mm
                    # bias add PSUM -> SBUF (bf16), then store this block
                    o_sb = opool.tile([F, HB, W], BF16, tag="o")
                    nc.vector.tensor_scalar_add(o_sb[:], ps[:], b_sb[:, 0:1])
                    nc.sync.dma_start(o_d[n][:, hb * HB : hb * HB + HB, :], o_sb[:])
    nc.compile()

    # Declare only the DMA queue group we use (SP HWDGE), and fewer rings:
    # NRT's per-execution queue init/reset work scales with declarations.
    nc.m.queues = [q for q in nc.m.queues if q.name == "qSPDynamicHW"]
    for q in nc.m.queues:
        q.num_queues = 8
    return nc


_NC_CACHE = None


def _run(x, weight, bias, **kwargs):
    global _NC_CACHE
    if _NC_CACHE is None:
        _NC_CACHE = _build_module()
    nc = _NC_CACHE

    xp = np.zeros((N, C, HP, WP), dtype=ml_dtypes.bfloat16)
    xp[:, :, 1 : 1 + H, 1 : 1 + W] = np.asarray(x, dtype=np.float32).astype(
        ml_dtypes.bfloat16
    )
    # per-block 18-row chunks: chunk hb = padded rows 16*hb .. 16*hb+18
    xc = np.stack([xp[:, :, 0:HC, :], xp[:, :, HB : HB + HC, :]], axis=1)
    # lhsT layout: w_pack[c, (ky*3+kx)*F + f] = weight[f, c, ky, kx]
    w_pack = np.ascontiguousarray(
        np.asarray(weight, dtype=np.float32).transpose(1, 2, 3, 0).reshape(C, 9 * F)
    ).astype(ml_dtypes.bfloat16)
    b2 = np.ascontiguousarray(np.asarray(bias, dtype=np.float32).reshape(F, 1))

    shards = xc.reshape(N_CORES, N_LOC, H // HB, C, HC, WP)
    in_maps = [{"x": shards[i], "w": w_pack, "b": b2} for i in range(N_CORES)]
    return run_bass_kernel_spmd(nc, in_maps, core_ids=list(range(N_CORES)), **kwargs)


def kernel(x: np.ndarray, weight: np.ndarray, bias: np.ndarray, **_) -> np.ndarray:
    res = _run(x, weight, bias)
    return np.concatenate(
        [res.results[i]["out"].astype(np.float32) for i in range(N_CORES)], axis=0
    )
